# revision 5
# baseline (speedup 1.0000x reference)
"""Trainium2 Bass kernel for nn_Decoder (3-stream attention decoder with
pointer-generator output), SPMD over 8 NeuronCores.

Sharding: data-parallel over batch B for embedding/attention/GRU/p_gen
(B/8 batch rows per core); tensor-parallel over the output vocab for the
out projection (V/8 vocab columns per core) with an in-NEFF AllGather of
[h_newT; ctxT; p_gen; ones] before the projection, plus a second tiny
AllGather for the global softmax denominator.

Math transforms (host-side, exact up to bf16 rounding):
 - energy: relu(cat([h,enc]) @ W.T + b) . v  ==  sum_h' sign(v_h') *
   relu(enc @ (W2*|v|).T + (h @ (W1*|v|).T + b*|v|)), so |v| folds into
   the weights and the per-h' bias rides the ACT Relu bias port.
 - softmax over the full vocab skips the max-subtraction (logits are
   bounded well inside fp32 exp range for this model).
 - the scatter-add of attention onto extended-vocab ids is applied as an
   exact float64 log-space fixup on host (only ~T touched ids per row).
"""

import numpy as np
import ml_dtypes

import concourse.bass as bass
import concourse.bacc as bacc
import concourse.mybir as mybir
import concourse.tile as tile
from concourse.bass_utils import run_bass_kernel_spmd

BF16 = ml_dtypes.bfloat16
EPS = 1e-12
N_CORES = 8

FULL_CFG = dict(T=512, H=512, B=64, V=50000, OOV=50)

_BUILD_CACHE = {}


def _cfg_derived(cfg):
    T, H, B, V = cfg["T"], cfg["H"], cfg["B"], cfg["V"]
    BC = B // N_CORES
    VS = V // N_CORES
    HC = H // 128
    TC = T // 128
    PK = 2 * H + 128
    KC = 2 * HC + 1
    NJ = [512] * (VS // 512) + ([VS % 512] if VS % 512 else [])
    assert B % N_CORES == 0 and V % N_CORES == 0 and H % 128 == 0 and T % 128 == 0
    assert BC <= 128
    return T, H, B, V, BC, VS, HC, TC, PK, KC, NJ


def _build(cfg_key):
    cfg = dict(cfg_key)
    T, H, B, V, BC, VS, HC, TC, PK, KC, NJ = _cfg_derived(cfg)
    fp32 = mybir.dt.float32
    bf16 = mybir.dt.bfloat16
    AX = mybir.AxisListType.X
    ALU = mybir.AluOpType
    AF = mybir.ActivationFunctionType

    nc = bacc.Bacc("TRN2", debug=False, num_devices=N_CORES)

    streams = ["src", "code", "ast"]
    ins = {}
    for s in streams:
        ins[f"encT_{s}"] = nc.dram_tensor(f"encT_{s}", [H, BC, T], bf16, kind="ExternalInput")
        ins[f"encN_{s}"] = nc.dram_tensor(f"encN_{s}", [BC, T, H], bf16, kind="ExternalInput")
        ins[f"w2vT_{s}"] = nc.dram_tensor(f"w2vT_{s}", [H, H], bf16, kind="ExternalInput")
        ins[f"w1vT_{s}"] = nc.dram_tensor(f"w1vT_{s}", [H, H], bf16, kind="ExternalInput")
        ins[f"bvT_{s}"] = nc.dram_tensor(f"bvT_{s}", [H, 1], fp32, kind="ExternalInput")
        ins[f"signT_{s}"] = nc.dram_tensor(f"signT_{s}", [H, 1], bf16, kind="ExternalInput")
    ins["h_prevT16"] = nc.dram_tensor("h_prevT16", [H, BC], bf16, kind="ExternalInput")
    ins["h_prev"] = nc.dram_tensor("h_prev", [BC, H], fp32, kind="ExternalInput")
    ins["embT16"] = nc.dram_tensor("embT16", [H, BC], bf16, kind="ExternalInput")
    ins["wihT"] = nc.dram_tensor("wihT", [2 * H + 128, 3 * H], bf16, kind="ExternalInput")
    ins["whhT"] = nc.dram_tensor("whhT", [H, 3 * H], bf16, kind="ExternalInput")
    ins["pgenWT"] = nc.dram_tensor("pgenWT", [3 * H + 128, 1], bf16, kind="ExternalInput")
    ins["owT"] = nc.dram_tensor("owT", [PK, VS], bf16, kind="ExternalInput")
    ins["ident"] = nc.dram_tensor("ident", [128, 128], fp32, kind="ExternalInput")

    out_final = nc.dram_tensor("final_s", [B, VS], fp32, kind="ExternalOutput")
    out_hnew = nc.dram_tensor("h_new_o", [BC, H], fp32, kind="ExternalOutput")
    out_w = {s: nc.dram_tensor(f"w_{s}_o", [BC, T], fp32, kind="ExternalOutput")
             for s in streams}
    out_pgen = nc.dram_tensor("pgen_o", [BC, 1], fp32, kind="ExternalOutput")

    with tile.TileContext(nc) as tc:
        with (
            tc.tile_pool(name="weights", bufs=1) as wpool,
            tc.tile_pool(name="psml", bufs=1) as psml,
            tc.tile_pool(name="ctxp", bufs=BC) as ctxp,
            tc.tile_pool(name="owt", bufs=min(len(NJ) * KC, 56)) as owp,
            tc.tile_pool(name="dram", bufs=1, space="DRAM") as dram,
        ):
            ident = wpool.tile([128, 128], fp32, tag="ident")
            nc.sync.dma_start(out=ident[:], in_=ins["ident"][:])

            def pe_t(out_ap, in_ap):
                p = in_ap.partition_size()
                nc.tensor.transpose(out_ap, in_ap, ident[:p, :p])

            h_prevT16 = wpool.tile([128, HC * BC], bf16, tag="hprevT")
            nc.sync.dma_start(
                out=h_prevT16[:].rearrange("p (k b) -> p k b", b=BC),
                in_=ins["h_prevT16"][:].rearrange("(k p) b -> p k b", p=128))
            embT16 = wpool.tile([128, HC * BC], bf16, tag="embT")
            nc.sync.dma_start(
                out=embT16[:].rearrange("p (k b) -> p k b", b=BC),
                in_=ins["embT16"][:].rearrange("(k p) b -> p k b", p=128))
            h_prev_sb = wpool.tile([BC, H], fp32, tag="hprev")
            nc.sync.dma_start(out=h_prev_sb[:], in_=ins["h_prev"][:])
            wih_sb = wpool.tile([128, KC * 3 * H], bf16, tag="wih")
            nc.sync.dma_start(
                out=wih_sb[:].rearrange("p (k h) -> p k h", h=3 * H),
                in_=ins["wihT"][:].rearrange("(k p) h -> p k h", p=128))
            whh_sb = wpool.tile([128, HC * 3 * H], bf16, tag="whh")
            nc.sync.dma_start(
                out=whh_sb[:].rearrange("p (k h) -> p k h", h=3 * H),
                in_=ins["whhT"][:].rearrange("(k p) h -> p k h", p=128))
            pgw = wpool.tile([128, 3 * HC + 1], bf16, tag="pgw")
            nc.sync.dma_start(
                out=pgw[:].rearrange("p (k o) -> p k o", o=1),
                in_=ins["pgenWT"][:].rearrange("(k p) o -> p k o", p=128))

            w2vT, w1vT, bvT, signT = {}, {}, {}, {}
            for s in streams:
                w2vT[s] = wpool.tile([128, HC * H], bf16, tag=f"w2vT_{s}", name=f"w2vT_{s}_sb")
                nc.sync.dma_start(
                    out=w2vT[s][:].rearrange("p (k h) -> p k h", h=H),
                    in_=ins[f"w2vT_{s}"][:].rearrange("(k p) h -> p k h", p=128))
                w1vT[s] = wpool.tile([128, HC * H], bf16, tag=f"w1vT_{s}", name=f"w1vT_{s}_sb")
                nc.sync.dma_start(
                    out=w1vT[s][:].rearrange("p (k h) -> p k h", h=H),
                    in_=ins[f"w1vT_{s}"][:].rearrange("(k p) h -> p k h", p=128))
                bvT[s] = wpool.tile([128, HC], fp32, tag=f"bvT_{s}", name=f"bvT_{s}_sb")
                nc.sync.dma_start(
                    out=bvT[s][:].rearrange("p (k o) -> p k o", o=1),
                    in_=ins[f"bvT_{s}"][:].rearrange("(k p) o -> p k o", p=128))
                signT[s] = wpool.tile([128, HC], bf16, tag=f"signT_{s}", name=f"signT_{s}_sb")
                nc.sync.dma_start(
                    out=signT[s][:].rearrange("p (k o) -> p k o", o=1),
                    in_=ins[f"signT_{s}"][:].rearrange("(k p) o -> p k o", p=128))

            ctx_acc = [psml.tile([1, H], fp32, tag=f"ctx_acc{b}", name=f"ctx_acc{b}") for b in range(BC)]
            w_out_sb = {}

            # ---------------- attention ----------------
            with (
                tc.tile_pool(name="etp", bufs=2 * HC + 2) as etp,
                tc.tile_pool(name="enp", bufs=2 * TC + 2) as enp,
                tc.tile_pool(name="vrp", bufs=HC + 4) as vrp,
                tc.tile_pool(name="att", bufs=2) as att,
                tc.tile_pool(name="pe", bufs=2, space="PSUM") as pe_pool,
                tc.tile_pool(name="psc", bufs=2, space="PSUM") as psc,
                tc.tile_pool(name="pcx", bufs=2, space="PSUM") as pcx,
                tc.tile_pool(name="ptp", bufs=2, space="PSUM") as ptp,
            ):
                for si, s in enumerate(streams):
                    # hpart = h_prev @ W1v.T   [BC, H] psum
                    hp_ps = ptp.tile([BC, H], fp32, tag="tp")
                    for k in range(HC):
                        nc.tensor.matmul(
                            hp_ps[:], h_prevT16[:, k * BC:(k + 1) * BC],
                            w1vT[s][:, k * H:(k + 1) * H],
                            start=(k == 0), stop=(k == HC - 1))
                    hp_sb = att.tile([BC, H], fp32, tag="hp_sb")
                    nc.vector.tensor_copy(hp_sb[:], hp_ps[:])
                    hpT = att.tile([128, HC * BC], fp32, tag="hpT")
                    for k in range(HC):
                        tp = ptp.tile([128, BC], fp32, tag="tp")
                        pe_t(tp[:], hp_sb[:, k * 128:(k + 1) * 128])
                        nc.vector.tensor_scalar(
                            out=hpT[:, k * BC:(k + 1) * BC], in0=tp[:],
                            scalar1=bvT[s][:, k:k + 1], scalar2=None,
                            op0=ALU.add)

                    sc_pack = att.tile([BC, T], fp32, tag="sc_pack")
                    for b in range(BC):
                        et = [etp.tile([128, T], bf16, tag="et", name=f"et{i}") for i in range(HC)]
                        for k in range(HC):
                            nc.sync.dma_start(
                                out=et[k][:],
                                in_=ins[f"encT_{s}"][k * 128:(k + 1) * 128, b, :])
                        vr = []
                        for hc in range(HC):
                            e_ps = pe_pool.tile([128, T], fp32, tag="e_ps")
                            for k in range(HC):
                                nc.tensor.matmul(
                                    e_ps[:],
                                    w2vT[s][:, k * H + hc * 128: k * H + (hc + 1) * 128],
                                    et[k][:],
                                    start=(k == 0), stop=(k == HC - 1))
                            v = vrp.tile([128, T], bf16, tag="vr")
                            nc.scalar.activation(
                                v[:], e_ps[:], AF.Relu,
                                bias=hpT[:, hc * BC + b: hc * BC + b + 1])
                            vr.append(v)
                        sc_ps = psc.tile([1, T], fp32, tag="sc")
                        for hc in range(HC):
                            nc.tensor.matmul(
                                sc_ps[:], signT[s][:, hc:hc + 1], vr[hc][:],
                                start=(hc == 0), stop=(hc == HC - 1))
                        sc_row = att.tile([1, T], fp32, tag=f"sc_row{b % 3}")
                        if b % 2 == 0:
                            nc.scalar.copy(sc_row[:], sc_ps[:])
                        else:
                            nc.vector.tensor_copy(sc_row[:], sc_ps[:])
                        nc.scalar.dma_start(out=sc_pack[b:b + 1, :], in_=sc_row[:])

                    # softmax over T (dense [BC, T])
                    stat = att.tile([BC, 4], fp32, tag="stat")
                    w_sb = att.tile([BC, T], fp32, tag=f"w_sb{si}")
                    nc.vector.tensor_reduce(out=stat[:, 0:1], in_=sc_pack[:],
                                            axis=AX, op=ALU.max, negate=True)
                    nc.scalar.activation(w_sb[:], sc_pack[:], AF.Exp,
                                         bias=stat[:, 0:1], accum_out=stat[:, 1:2])
                    nc.vector.reciprocal(stat[:, 2:3], stat[:, 1:2])
                    nc.vector.tensor_scalar_mul(w_sb[:], w_sb[:], stat[:, 2:3])
                    nc.scalar.dma_start(out=out_w[s][:], in_=w_sb[:])
                    w_out_sb[s] = w_sb

                    # wT chunks [128, BC] bf16, scaled for ctx accumulation
                    wt16 = att.tile([128, TC * BC], bf16, tag="wt16")
                    scale = 0.5 if s in ("src", "code") else 1.0
                    for k in range(TC):
                        tp = ptp.tile([128, BC], fp32, tag="tp")
                        pe_t(tp[:], w_sb[:, k * 128:(k + 1) * 128])
                        nc.vector.tensor_scalar_mul(
                            wt16[:, k * BC:(k + 1) * BC], tp[:], scale)

                    # ctx_b += sum_t w[b,t] * encN[b,t,:]
                    for b in range(BC):
                        en = [enp.tile([128, H], bf16, tag="en", name=f"en{i}") for i in range(TC)]
                        for k in range(TC):
                            nc.sync.dma_start(
                                out=en[k][:],
                                in_=ins[f"encN_{s}"][b, k * 128:(k + 1) * 128, :])
                        cx_ps = pcx.tile([1, H], fp32, tag="cx")
                        for k in range(TC):
                            nc.tensor.matmul(
                                cx_ps[:], wt16[:, k * BC + b:k * BC + b + 1],
                                en[k][:],
                                start=(k == 0), stop=(k == TC - 1))
                        if si == 0:
                            nc.vector.tensor_copy(ctx_acc[b][:], cx_ps[:])
                        else:
                            nc.vector.tensor_tensor(
                                out=ctx_acc[b][:], in0=cx_ps[:],
                                in1=ctx_acc[b][:], op=ALU.add)

            # pack ctx rows -> dense [BC, H]
            ctx_pack = psml.tile([BC, H], fp32, tag="ctx_pack")
            for b in range(BC):
                nc.scalar.dma_start(out=ctx_pack[b:b + 1, :], in_=ctx_acc[b][:])

            # ---------------- GRU + p_gen ----------------
            ctxT_sb = psml.tile([128, HC * BC], fp32, tag="ctxT_sb")
            ctxT16 = psml.tile([128, HC * BC], bf16, tag="ctxT16")
            hnT32 = psml.tile([128, HC * BC], fp32, tag="hnT32")
            with (
                tc.tile_pool(name="gru", bufs=1) as gru,
                tc.tile_pool(name="pg1", bufs=1, space="PSUM") as pg1,
                tc.tile_pool(name="ptp2", bufs=2, space="PSUM") as ptp2,
            ):
                for k in range(HC):
                    tp = ptp2.tile([128, BC], fp32, tag="tp2")
                    pe_t(tp[:], ctx_pack[:, k * 128:(k + 1) * 128])
                    nc.vector.tensor_copy(ctxT_sb[:, k * BC:(k + 1) * BC], tp[:])
                    nc.vector.tensor_copy(ctxT16[:, k * BC:(k + 1) * BC], tp[:])

                ones16 = psml.tile([128, BC], bf16, tag="ones16")
                nc.vector.memset(ones16[:], 0.0)
                nc.vector.memset(ones16[0:1, :], 1.0)

                xT = ([(embT16, k * BC) for k in range(HC)]
                      + [(ctxT16, k * BC) for k in range(HC)]
                      + [(ones16, 0)])
                gr_ps = pg1.tile([BC, H], fp32, tag="gr")
                gz_ps = pg1.tile([BC, H], fp32, tag="gz")
                gin_ps = pg1.tile([BC, H], fp32, tag="gin")
                ghn_ps = pg1.tile([BC, H], fp32, tag="ghn")
                for part, ps in ((gr_ps, 0), (gz_ps, 1), (gin_ps, 2)):
                    for k in range(KC):
                        t_, co = xT[k]
                        nc.tensor.matmul(
                            part[:], t_[:, co:co + BC],
                            wih_sb[:, k * 3 * H + ps * H: k * 3 * H + (ps + 1) * H],
                            start=(k == 0), stop=False, skip_group_check=True)
                for part, ps in ((gr_ps, 0), (gz_ps, 1), (ghn_ps, 2)):
                    first = part is ghn_ps
                    for k in range(HC):
                        nc.tensor.matmul(
                            part[:], h_prevT16[:, k * BC:(k + 1) * BC],
                            whh_sb[:, k * 3 * H + ps * H: k * 3 * H + (ps + 1) * H],
                            start=(first and k == 0), stop=(k == HC - 1),
                            skip_group_check=True)

                def sigmoid_from(ps_t, out_t, tag):
                    e = gru.tile(list(out_t.shape), fp32, tag=tag + "_e")
                    nc.scalar.activation(e[:], ps_t[:], AF.Exp, scale=-1.0)
                    nc.vector.tensor_scalar_add(e[:], e[:], 1.0)
                    nc.vector.reciprocal(out_t[:], e[:])

                r_sb = gru.tile([BC, H], fp32, tag="r_sb")
                z_sb = gru.tile([BC, H], fp32, tag="z_sb")
                sigmoid_from(gr_ps, r_sb, "r")
                sigmoid_from(gz_ps, z_sb, "z")
                rh = gru.tile([BC, H], fp32, tag="rh")
                nc.vector.tensor_tensor(out=rh[:], in0=ghn_ps[:], in1=r_sb[:], op=ALU.mult)
                pre_n = gru.tile([BC, H], fp32, tag="pre_n")
                nc.vector.tensor_tensor(out=pre_n[:], in0=gin_ps[:], in1=rh[:], op=ALU.add)
                n_sb = gru.tile([BC, H], fp32, tag="n_sb")
                nc.scalar.activation(n_sb[:], pre_n[:], AF.Tanh)
                zn = gru.tile([BC, H], fp32, tag="zn")
                nc.vector.tensor_tensor(out=zn[:], in0=n_sb[:], in1=z_sb[:], op=ALU.mult)
                zh = gru.tile([BC, H], fp32, tag="zh")
                nc.vector.tensor_tensor(out=zh[:], in0=h_prev_sb[:], in1=z_sb[:], op=ALU.mult)
                hnew = gru.tile([BC, H], fp32, tag="hnew")
                nc.vector.tensor_tensor(out=hnew[:], in0=n_sb[:], in1=zn[:], op=ALU.subtract)
                nc.vector.tensor_tensor(out=hnew[:], in0=hnew[:], in1=zh[:], op=ALU.add)
                nc.scalar.dma_start(out=out_hnew[:], in_=hnew[:])

                catT = ([(ctxT16, k * BC) for k in range(HC)]
                        + [(h_prevT16, k * BC) for k in range(HC)]
                        + [(embT16, k * BC) for k in range(HC)]
                        + [(ones16, 0)])
                pg_ps = pg1.tile([BC, 1], fp32, tag="pg")
                for k, (t_, co) in enumerate(catT):
                    nc.tensor.matmul(pg_ps[:], t_[:, co:co + BC], pgw[:, k:k + 1],
                                     start=(k == 0), stop=(k == len(catT) - 1))
                pg_sb = psml.tile([BC, 1], fp32, tag="pg_sb")
                sigmoid_from(pg_ps, pg_sb, "pg")
                nc.scalar.dma_start(out=out_pgen[:], in_=pg_sb[:])

                for k in range(HC):
                    tp = ptp2.tile([128, BC], fp32, tag="tp2")
                    pe_t(tp[:], hnew[:, k * 128:(k + 1) * 128])
                    nc.vector.tensor_copy(hnT32[:, k * BC:(k + 1) * BC], tp[:])

                pgT_ps = ptp2.tile([1, BC], fp32, tag="tp2")
                pe_t(pgT_ps[:], pg_sb[:])
                pgT_sb = psml.tile([1, BC], fp32, tag="pgT_sb")
                nc.vector.tensor_copy(pgT_sb[:], pgT_ps[:])

            ones_row = psml.tile([1, BC], fp32, tag="ones_row")
            nc.vector.memset(ones_row[:], 1.0)
            zrows = psml.tile([126, BC], fp32, tag="zrows")
            nc.vector.memset(zrows[:], 0.0)

            pkg = dram.tile([PK, BC], fp32)
            for k in range(HC):
                nc.scalar.dma_start(out=pkg[k * 128:(k + 1) * 128, :],
                                    in_=hnT32[:, k * BC:(k + 1) * BC])
                nc.scalar.dma_start(out=pkg[H + k * 128:H + (k + 1) * 128, :],
                                    in_=ctxT_sb[:, k * BC:(k + 1) * BC])
            nc.scalar.dma_start(out=pkg[2 * H:2 * H + 1, :], in_=pgT_sb[:])
            nc.scalar.dma_start(out=pkg[2 * H + 1:2 * H + 2, :], in_=ones_row[:])
            nc.scalar.dma_start(out=pkg[2 * H + 2:PK, :], in_=zrows[:])

            gath = dram.tile([N_CORES * PK, BC], fp32)
            nc.gpsimd.collective_compute(
                "AllGather", mybir.AluOpType.bypass,
                replica_groups=[list(range(N_CORES))],
                ins=[pkg.opt()], outs=[gath.opt()])

            # ---------------- vocab projection ----------------
            with (
                tc.tile_pool(name="xcp", bufs=KC) as xcp,
                tc.tile_pool(name="xct", bufs=3) as xct,
                tc.tile_pool(name="esb", bufs=len(NJ)) as esb,
                tc.tile_pool(name="fg", bufs=3) as fg,
                tc.tile_pool(name="plg", bufs=2, space="PSUM") as plg,
                tc.tile_pool(name="ptp3", bufs=1, space="PSUM") as ptp3,
            ):
                gv3 = gath[:].rearrange("(c r) b -> r c b", c=N_CORES)
                xc16 = []
                for k in range(KC):
                    xf = xct.tile([128, B], fp32, tag="xc32")
                    nc.sync.dma_start(out=xf[:].rearrange("p (c b) -> p c b", b=BC),
                                      in_=gv3[k * 128:(k + 1) * 128, :, :])
                    x6 = xcp.tile([128, B], bf16, tag="xc16")
                    nc.vector.tensor_copy(x6[:], xf[:])
                    xc16.append(x6)
                pgrow = psml.tile([1, B], fp32, tag="pgrow")
                nc.sync.dma_start(out=pgrow[:].rearrange("p (c b) -> p c b", b=BC),
                                  in_=gv3[2 * H:2 * H + 1, :, :])
                pga_ps = ptp3.tile([B, 1], fp32, tag="pga")
                pe_t(pga_ps[:], pgrow[:])
                pg_all = psml.tile([B, 1], fp32, tag="pg_all")
                nc.vector.tensor_copy(pg_all[:], pga_ps[:])

                sums = psml.tile([B, len(NJ) + 1], fp32, tag="sums")
                e_sb = []
                for j, njw in enumerate(NJ):
                    lg_ps = plg.tile([B, 512], fp32, tag="lg")
                    for k in range(KC):
                        ow = owp.tile([128, 512], bf16, tag="ow")
                        nc.sync.dma_start(
                            out=ow[:, :njw],
                            in_=ins["owT"][k * 128:(k + 1) * 128, j * 512:j * 512 + njw])
                        nc.tensor.matmul(lg_ps[:, :njw], xc16[k][:], ow[:, :njw],
                                         start=(k == 0), stop=(k == KC - 1))
                    e = esb.tile([B, 512], fp32, tag="e_sb")
                    nc.scalar.activation(e[:, :njw], lg_ps[:, :njw], AF.Exp,
                                         accum_out=sums[:, j:j + 1])
                    e_sb.append(e)
                nc.vector.tensor_reduce(out=sums[:, len(NJ):len(NJ) + 1],
                                        in_=sums[:, :len(NJ)], axis=AX, op=ALU.add)
                sb_l = dram.tile([B, 1], fp32)
                nc.scalar.dma_start(out=sb_l[:], in_=sums[:, len(NJ):len(NJ) + 1])
                sgath = dram.tile([N_CORES * B, 1], fp32)
                nc.gpsimd.collective_compute(
                    "AllGather", mybir.AluOpType.bypass,
                    replica_groups=[list(range(N_CORES))],
                    ins=[sb_l.opt()], outs=[sgath.opt()])
                sloc = psml.tile([B, N_CORES], fp32, tag="sloc")
                nc.sync.dma_start(
                    out=sloc[:].rearrange("b (c o) -> b c o", o=1),
                    in_=sgath[:].rearrange("(c b) o -> b c o", c=N_CORES))
                gsum = psml.tile([B, 3], fp32, tag="gsum")
                nc.vector.tensor_reduce(out=gsum[:, 0:1], in_=sloc[:], axis=AX, op=ALU.add)
                nc.vector.reciprocal(gsum[:, 1:2], gsum[:, 0:1])
                nc.vector.tensor_tensor(out=gsum[:, 2:3], in0=gsum[:, 1:2],
                                        in1=pg_all[:], op=ALU.mult)
                for j, njw in enumerate(NJ):
                    f = fg.tile([B, 512], fp32, tag="f_sb")
                    nc.vector.tensor_scalar(
                        out=f[:, :njw], in0=e_sb[j][:, :njw],
                        scalar1=gsum[:, 2:3], scalar2=float(EPS),
                        op0=ALU.mult, op1=ALU.add)
                    g = fg.tile([B, 512], fp32, tag="g_sb")
                    nc.scalar.activation(g[:, :njw], f[:, :njw], AF.Ln)
                    nc.sync.dma_start(out=out_final[:, j * 512:j * 512 + njw],
                                      in_=g[:, :njw])

    nc.finalize()
    return nc


def _host_prep(inputs, cfg):
    T, H, B, V, BC, VS, HC, TC, PK, KC, NJ = _cfg_derived(cfg)
    f32 = lambda a: np.ascontiguousarray(a, dtype=np.float32)
    b16 = lambda a: np.ascontiguousarray(np.asarray(a, dtype=np.float32).astype(BF16))

    emb_tab = np.asarray(inputs["embedding"], dtype=np.float32)
    idx = np.asarray(inputs["inputs"]).astype(np.int64)
    embedded = emb_tab[idx]
    h_prev = np.asarray(inputs["last_hidden"], dtype=np.float32)[0]

    stream_in = {"src": "source_outputs", "code": "code_outputs", "ast": "ast_outputs"}
    per_stream = {}
    for s, key in stream_in.items():
        W = np.asarray(inputs[f"{s}_attn_W"], dtype=np.float32)
        bvec = np.asarray(inputs[f"{s}_attn_b"], dtype=np.float32)
        v = np.asarray(inputs[f"{s}_v"], dtype=np.float32)
        av = np.abs(v)
        W1, W2 = W[:, :H], W[:, H:]
        enc16 = np.asarray(inputs[key], dtype=np.float32).astype(BF16)  # [T,B,H]
        per_stream[s] = dict(
            w2vT=b16((W2 * av[:, None]).T),
            w1vT=b16((W1 * av[:, None]).T),
            bvT=f32((bvec * av)[:, None]),
            signT=b16(np.sign(v)[:, None]),
            encT=enc16.transpose(2, 1, 0),
            encN=enc16.transpose(1, 0, 2),
        )

    w_ih = np.asarray(inputs["gru_w_ih"], dtype=np.float32)
    w_hh = np.asarray(inputs["gru_w_hh"], dtype=np.float32)
    b_ih = np.asarray(inputs["gru_b_ih"], dtype=np.float32)
    b_hh = np.asarray(inputs["gru_b_hh"], dtype=np.float32)
    wihT = np.zeros((2 * H + 128, 3 * H), dtype=BF16)
    wihT[:2 * H] = w_ih.T.astype(BF16)
    wihT[2 * H] = (b_ih + b_hh).astype(BF16)
    whhT = b16(w_hh.T)

    pgen_W = np.asarray(inputs["pgen_W"], dtype=np.float32)
    pgen_b = np.asarray(inputs["pgen_b"], dtype=np.float32)
    pgenWT = np.zeros((3 * H + 128, 1), dtype=BF16)
    pgenWT[:3 * H, 0] = pgen_W[0].astype(BF16)
    pgenWT[3 * H, 0] = BF16(pgen_b[0])

    out_W = np.asarray(inputs["out_W"], dtype=np.float32)
    out_b = np.asarray(inputs["out_b"], dtype=np.float32)
    owT_full = np.zeros((PK, V), dtype=BF16)
    owT_full[:2 * H] = out_W.T.astype(BF16)
    owT_full[2 * H + 1] = out_b.astype(BF16)

    ident = np.eye(128, dtype=np.float32)

    in_maps = []
    for c in range(N_CORES):
        bs = slice(c * BC, (c + 1) * BC)
        m = dict(
            h_prevT16=b16(h_prev[bs].T),
            h_prev=f32(h_prev[bs]),
            embT16=b16(embedded[bs].T),
            wihT=wihT, whhT=whhT, pgenWT=pgenWT,
            owT=np.ascontiguousarray(owT_full[:, c * VS:(c + 1) * VS]),
            ident=ident,
        )
        for s in stream_in:
            ps = per_stream[s]
            m[f"encT_{s}"] = np.ascontiguousarray(ps["encT"][:, bs, :])
            m[f"encN_{s}"] = np.ascontiguousarray(ps["encN"][bs])
            m[f"w2vT_{s}"] = ps["w2vT"]
            m[f"w1vT_{s}"] = ps["w1vT"]
            m[f"bvT_{s}"] = ps["bvT"]
            m[f"signT_{s}"] = ps["signT"]
        in_maps.append(m)
    return in_maps


def _run(inputs, cfg, **run_kwargs):
    key = tuple(sorted(cfg.items()))
    if key not in _BUILD_CACHE:
        _BUILD_CACHE[key] = _build(key)
    nc = _BUILD_CACHE[key]
    in_maps = _host_prep(inputs, cfg)
    return run_bass_kernel_spmd(nc, in_maps, core_ids=list(range(N_CORES)),
                                **run_kwargs)


def _assemble(inputs, res, cfg):
    T, H, B, V, BC, VS, HC, TC, PK, KC, NJ = _cfg_derived(cfg)
    OOV = cfg["OOV"]
    rs = res.results
    hidden = np.concatenate([rs[c]["h_new_o"] for c in range(N_CORES)], axis=0)[None]
    ws = {s: np.concatenate([rs[c][f"w_{s}_o"] for c in range(N_CORES)],
                            axis=0)[:, None, :]
          for s in ("src", "code", "ast")}
    p_gen = np.concatenate([rs[c]["pgen_o"] for c in range(N_CORES)], axis=0)

    base = np.concatenate([rs[c]["final_s"] for c in range(N_CORES)], axis=1)
    final = np.full((B, V + OOV), np.log(EPS), dtype=np.float64)
    final[:, :V] = base.astype(np.float64)

    attn = (1.0 - p_gen.astype(np.float64)) * ws["src"][:, 0, :].astype(np.float64)
    ext = np.asarray(inputs["extend_source_batch"]).astype(np.int64)
    add = np.zeros((B, V + OOV), dtype=np.float64)
    np.add.at(add, (np.arange(B)[:, None], ext), attn)
    touched = add != 0
    final[touched] = np.log(np.exp(final[touched]) + add[touched])
    final = final.astype(np.float32)

    return (final, hidden.astype(np.float32),
            ws["src"].astype(np.float32), ws["code"].astype(np.float32),
            ws["ast"].astype(np.float32), p_gen.astype(np.float32))


def kernel(**inputs):
    cfg = dict(FULL_CFG)
    res = _run(inputs, cfg)
    return _assemble(inputs, res, cfg)
